# revision 17
# baseline (speedup 1.0000x reference)
"""RBF-kernel attention (dense_transformer) on 8 TRN2 NeuronCores.

Reference computation (B=1, S=4096, D=768, H=12, Dh=64):
    q,k,v = x@Wq, x@Wk, x@Wv               (per-head split)
    dist  = ||q_s - k_t||^2
    scores= exp(-gamma_h/8 * dist)
    out   = (scores @ v) merged @ Wo

Sharding: 8-way data parallel over query rows (512 rows/core).  Each core
computes its local K/V shard + per-head k-norms, all-gathers an augmented
K (rows: [k(64); kn_hi; kn_lo]) and V across cores, then computes the
full distance matrix for its queries with a single 80-deep matmul per
tile (rows 66:80 = [1;1;0*12] are constant and reconstructed on-chip):
    dist[t,s] = kaug[:,t] . qaug[:,s],  qaug = [-2q; 1; 1; qn_hi; qn_lo; 0]
(contraction padded to 80: K%16 != 0 streams at half rate on the PE).

v3 schedule: the first collective cannot execute before ~65us after
launch (fixed CC warmup) and each mesh costs ~10us fixed + bytes/190GB/s,
so the gathers are packed into 7 parts ordered by phase-B deadline:
m0={kaug pair0}, m_i={V pair i-1, kaug pair i} (i=1..5), m6={V pair5}.
All Q projections run in the otherwise-idle pre-gather window.  Phase B
is software-pipelined by head: dist/exp for head h interleaves with
attn@V for head h-1, so the V gather deadline trails the kaug deadline
by a full head slot (~16us).  The output projection is interleaved into
later head slots (SBUF accumulation via DVE adds off a single scratch
PSUM bank), so there is no serial projection tail.  The two heads of a
pair share one PSUM bank for attn@V output (odd head at partition
offset 64 via tile_position).  Norm matmuls run in bf16 (fp32 operands
cost 4 cycles/row on the PE).  exp runs on the scalar engine straight
out of PSUM with the per-head scale folded in, over 1536-column groups
to amortize ACT overhead.  attn@V is computed transposed (out^T[d,s])
so no on-chip transposes are needed, and the final Wo matmul emits the
core's output slice transposed ([768, 512]); the host transposes and
concatenates.  All TensorE-facing data is bf16 (fp32 PSUM accumulation);
k/q norms get a hi+lo bf16 split so the exponent stays fp32-accurate.
"""

import numpy as np
import ml_dtypes

N_CORES = 8
S = 4096          # sequence length
D = 768           # embed dim
H = 12            # heads
DH = 64           # head dim
SL = S // N_CORES # query rows per core (512)
P = 128
KC = D // P       # contraction chunks for projections (6)
NAUG = DH + 4     # meaningful aug rows (68)
AUG = 80          # padded to mult-of-16: K%16!=0 matmuls stream at half rate
GAUG = DH + 2     # gathered aug rows (66): k + kn_hi + kn_lo
SCALE = 1.0 / np.sqrt(DH)
NPAIR = H // 2    # 6 head pairs

_BF16 = ml_dtypes.bfloat16


def build(neg_a):
    """Build the SPMD Bass graph. neg_a: list of 12 floats (-gamma[h]*SCALE)."""
    import concourse.bass as bass  # noqa: F401
    import concourse.mybir as mybir
    import concourse.tile as tile
    from concourse import bacc

    fb = mybir.dt.bfloat16
    f32 = mybir.dt.float32

    nc = bacc.Bacc("TRN2", target_bir_lowering=False, debug=False,
                   num_devices=N_CORES)

    xT = nc.dram_tensor("xT", [D, SL], fb, kind="ExternalInput").ap()
    wq = nc.dram_tensor("wq", [D, D], fb, kind="ExternalInput").ap()
    wk = nc.dram_tensor("wk", [D, D], fb, kind="ExternalInput").ap()
    wv = nc.dram_tensor("wv", [D, D], fb, kind="ExternalInput").ap()
    wo = nc.dram_tensor("wo", [D, D], fb, kind="ExternalInput").ap()
    outT = nc.dram_tensor("outT", [D, SL], f32, kind="ExternalOutput").ap()

    # 7 sub-1MB gathers ordered by phase-B deadline (V pair0 rides with
    # kaug pair0 in m0 -- the ~65us collective-warmup floor hides the
    # later V-projection finish, and head0's attn@V then never stalls):
    #   part 0      = kaug pair0 + V pair0        (266KB send)
    #   part 1      = kaug pair1                  (135KB send)
    #   part 2..5   = V pair p-1 + kaug pair p    (266KB send)
    #   part 6      = V pair5                     (131KB send)
    KSZ = 2 * GAUG * SL
    VSZ = SL * P
    PART_SZ = [KSZ + VSZ, KSZ] + [KSZ + VSZ] * 4 + [VSZ]
    # V part/offset for head pair hp
    VPART = [(0, KSZ)] + [(hp + 1, KSZ) for hp in range(1, 5)] + [(6, 0)]
    fsend = [nc.dram_tensor(f"fsend{p}", [PART_SZ[p]], fb) for p in range(7)]
    fg = [nc.dram_tensor(f"fg{p}", [N_CORES * PART_SZ[p]], fb,
                         addr_space="Shared") for p in range(7)]
    rg = [list(range(N_CORES))]

    def ksend2d(h, row, nrows):
        base = (h % 2) * GAUG * SL + row * SL
        return fsend[h // 2][base:base + nrows * SL].rearrange(
            "(a b) -> a b", b=SL)

    def vsend2d(hp, tt):
        prt, off = VPART[hp]
        base = off + tt * P * P
        return fsend[prt][base:base + P * P].rearrange("(a b) -> a b", b=P)

    def kg_src(p, c):
        """Gathered kaug of pair p, core c: [GAUG, 2, SL]."""
        base = c * PART_SZ[p]
        return fg[p][base:base + KSZ].rearrange(
            "(h a b) -> a h b", h=2, b=SL)

    def vg_src(hp, c):
        """Gathered V cols of pair hp, keys c*512..: [128, 4, 128]."""
        prt, off = VPART[hp]
        base = c * PART_SZ[prt] + off
        return fg[prt][base:base + VSZ].rearrange(
            "(j p c) -> p j c", j=4, p=P, c=P)

    groups = [list(range(g * 3, min(32, g * 3 + 3)))
              for g in range((32 + 2) // 3)]
    NG = len(groups)  # 11

    with tile.TileContext(nc) as tc:
        with tc.tile_pool(name="persist", bufs=1) as pp:
            xT_sb = [pp.tile([P, SL], fb, name=f"xT_sb{k}") for k in range(KC)]
            wo_sb = [pp.tile([P, D], fb, name=f"wo_sb{k}") for k in range(KC)]
            qaug = [pp.tile([AUG, SL], fb, name=f"qaug{h}") for h in range(H)]
            # 2-pair rings: gathered kaug [80, 2, SL] (rows 66:80 constant,
            # initialized once) and gathered V [128, 4, 128]
            kgr = [pp.tile([AUG, 2, SL], fb, name=f"kgr{s}") for s in range(16)]
            vPr = [pp.tile([P, 4, P], fb, name=f"vPr{s}") for s in range(16)]
            ot_sb = [pp.tile([P, SL], fb, name=f"ot_sb{m}") for m in range(KC)]
            acc_sb = [pp.tile([P, SL], f32, name=f"acc_sb{m}")
                      for m in range(KC)]
            hsel = pp.tile([P, 2], fb, name="hsel")

            # K-path input loads first: the kaug gathers gate phase B.
            for k in range(KC):
                nc.scalar.dma_start(xT_sb[k][:], xT[k * P:(k + 1) * P, :])

            # head-pair selector for partition-sum via matmul:
            # col j sums partitions j*64..j*64+63  (bf16: fp32 matmuls
            # stream at 1/4 rate on the PE)
            nc.vector.memset(hsel[:], 0.0)
            nc.vector.memset(hsel[0:DH, 0:1], 1.0)
            nc.vector.memset(hsel[DH:P, 1:2], 1.0)

            ones_sb = pp.tile([2, SL], fb, name="ones_sb")
            nc.vector.memset(ones_sb[:], 1.0)
            zeros_sb = pp.tile([AUG - NAUG, SL], fb, name="zeros_sb")
            nc.vector.memset(zeros_sb[:], 0.0)
            # kaug rows 66:80 = [1,1,0*12] in both head slots: one-time init
            onz_sb = pp.tile([AUG - GAUG, 2 * SL], fb, name="onz_sb")
            nc.vector.memset(onz_sb[:], 0.0)
            nc.vector.memset(onz_sb[0:2, :], 1.0)

            def project_T(w_sb, dt, pool):
                """psum[128, SL] = (W^T x^T) rows dt*128..+128."""
                ps = pool.tile([P, SL], f32, name=f"projT{dt}", tag="scr")
                for k in range(KC):
                    nc.tensor.matmul(ps[:], lhsT=w_sb[k][:, dt * P:(dt + 1) * P],
                                     rhs=xT_sb[k][:], start=(k == 0),
                                     stop=(k == KC - 1))
                return ps

            def norms(ps_bf, dt, tag, wpool, npool):
                """hi/lo bf16 split of per-head sum of squares.

                Returns [34, SL] tile: rows 0:2 = hi (head pair), rows
                32:34 = lo -- 32-aligned so compute engines may write both,
                and nhl[half::32] DMAs one head's (hi, lo) pair at once.
                """
                sq = wpool.tile([P, SL], fb, name=f"sq_{tag}{dt}",
                                tag=f"sq{tag}")
                nc.vector.tensor_mul(sq[:], ps_bf[:], ps_bf[:])
                nps = npool.tile([2, SL], f32, name=f"n_{tag}{dt}", tag="nrm")
                nc.tensor.matmul(nps[:], lhsT=hsel[:], rhs=sq[:],
                                 start=True, stop=True)
                nhl = wpool.tile([34, SL], fb, name=f"nhl_{tag}{dt}",
                                 tag=f"nhl{tag}")
                nc.vector.tensor_copy(nhl[0:2, :], nps[:])
                nc.vector.tensor_sub(nhl[32:34, :], nps[:], nhl[0:2, :])
                return nhl

            # ---------------- phase A -------------------------------------
            with tc.tile_pool(name="psA", bufs=3, space="PSUM") as psA, \
                 tc.tile_pool(name="psN", bufs=2, space="PSUM") as psN, \
                 tc.tile_pool(name="workA", bufs=3) as wa:

                wk_sb = [wa.tile([P, D], fb, name=f"wk_sb{k}", bufs=1)
                         for k in range(KC)]
                wv_sb = [wa.tile([P, D], fb, name=f"wv_sb{k}", bufs=1)
                         for k in range(KC)]
                wq_sb = [wa.tile([P, D], fb, name=f"wq_sb{k}", bufs=1)
                         for k in range(KC)]
                # wk dt0/dt1 column pieces first, then wv (V projections
                # start right after K dt0/dt1), then wq, then the rest
                for dt in range(2):
                    for k in range(KC):
                        nc.sync.dma_start(
                            wk_sb[k][:, dt * P:(dt + 1) * P],
                            wk[k * P:(k + 1) * P, dt * P:(dt + 1) * P])
                for k in range(KC):
                    nc.sync.dma_start(wv_sb[k][:], wv[k * P:(k + 1) * P, :])
                for k in range(KC):
                    nc.sync.dma_start(wq_sb[k][:], wq[k * P:(k + 1) * P, :])
                for dt in range(2, KC):
                    for k in range(KC):
                        nc.sync.dma_start(
                            wk_sb[k][:, dt * P:(dt + 1) * P],
                            wk[k * P:(k + 1) * P, dt * P:(dt + 1) * P])
                for k in range(KC):
                    nc.sync.dma_start(wo_sb[k][:], wo[k * P:(k + 1) * P, :])

                def emit_k_dt(dt):
                    """K projection chunk dt -> kaug sends (66 rows)."""
                    ps = project_T(wk_sb, dt, psA)
                    ktb = wa.tile([P, SL], fb, name=f"ktb{dt}", tag="ktb")
                    nc.vector.tensor_copy(ktb[:], ps[:])
                    nhl = norms(ktb, dt, "k", wa, psN)
                    for half in range(2):
                        h = 2 * dt + half
                        nc.scalar.dma_start(ksend2d(h, 0, DH),
                                            ktb[half * DH:(half + 1) * DH, :])
                        nc.scalar.dma_start(ksend2d(h, DH, 2),
                                            nhl[half:34:32, :])

                def fire(p):
                    nc.gpsimd.collective_compute(
                        "AllGather", mybir.AluOpType.bypass,
                        ins=[fsend[p][:]], outs=[fg[p][:]], replica_groups=rg)

                emit_k_dt(0)
                emit_k_dt(1)

                # V local (natural layout), sends per head-pair column block
                for tt in range(SL // P):
                    vloc = wa.tile([P, D], fb, name=f"vloc{tt}", tag="vloc")
                    for nh in range(2):
                        ps = psA.tile([P, 384], f32, name=f"vps{tt}_{nh}",
                                      tag="scr")
                        for k in range(KC):
                            nc.tensor.matmul(
                                ps[:], lhsT=xT_sb[k][:, tt * P:(tt + 1) * P],
                                rhs=wv_sb[k][:, nh * 384:(nh + 1) * 384],
                                start=(k == 0), stop=(k == KC - 1))
                        nc.vector.tensor_copy(vloc[:, nh * 384:(nh + 1) * 384],
                                              ps[:])
                    for hp in range(NPAIR):
                        nc.scalar.dma_start(vsend2d(hp, tt),
                                            vloc[:, hp * P:(hp + 1) * P])

                fire(0)
                fire(1)
                emit_k_dt(2)
                fire(2)
                emit_k_dt(3)
                fire(3)
                emit_k_dt(4)
                fire(4)
                emit_k_dt(5)
                fire(5)
                fire(6)

                # one-time kaug constant rows (66:80 = [1,1,0*12]); on the
                # sync queue so the overloaded scalar queue can't delay them
                for s in range(16):
                    nc.sync.dma_start(
                        kgr[s][GAUG:AUG, :, :],
                        onz_sb[:].rearrange("a (h b) -> a h b", h=2))

                # all Q projections fill the pre-gather idle window.  The
                # qtb/nhl tags are distinct from the K side so buffer reuse
                # never makes the Q pipeline wait on K send DMAs.
                for dt in range(KC):
                    ps = project_T(wq_sb, dt, psA)
                    qtb = wa.tile([P, SL], fb, name=f"qtb{dt}", tag="qtb")
                    nc.vector.tensor_copy(qtb[:], ps[:])
                    nhl = norms(qtb, dt, "q", wa, psN)
                    for half in range(2):
                        h = 2 * dt + half
                        qa = qaug[h]
                        nc.vector.tensor_scalar_mul(
                            qa[0:DH, :], qtb[half * DH:(half + 1) * DH, :],
                            -2.0)
                        # rows 64-67 ([1;1;qn_hi;qn_lo]) + zero pad via DMA:
                        # partition offsets 65..67 aren't 32-aligned for
                        # compute engines
                        nc.sync.dma_start(qa[DH:DH + 2, :], ones_sb[:])
                        nc.sync.dma_start(qa[DH + 2:DH + 4, :],
                                          nhl[half:34:32, :])
                        nc.sync.dma_start(qa[NAUG:AUG, :], zeros_sb[:])

            # ---------------- phase B: scores + attn@V --------------------
            # software-pipelined by head: head h's dist/exp interleaves
            # with head h-1's attn@V.  PSUM: 2*3 (dist) + 1 (o_ps pair-
            # packed) + 1 scratch (out-proj) = 8 banks
            with tc.tile_pool(name="psD", bufs=2, space="PSUM") as psD, \
                 tc.tile_pool(name="psO", bufs=1, space="PSUM") as psO, \
                 tc.tile_pool(name="psS", bufs=1, space="PSUM") as psS, \
                 tc.tile_pool(name="scp", bufs=16) as scp:

                def kg_load(p):
                    for c in range(N_CORES):
                        s = (p % 2) * 8 + c
                        nc.sync.dma_start(kgr[s][0:GAUG, :, :], kg_src(p, c))

                def vP_load(p):
                    for c in range(N_CORES):
                        s = (p % 2) * 8 + c
                        nc.sync.dma_start(vPr[s][:], vg_src(p, c))

                def dist_mm(h, dist, idx, j):
                    s = ((h // 2) % 2) * 8 + j // 4
                    nc.tensor.matmul(
                        dist[:, idx * SL:(idx + 1) * SL],
                        lhsT=kgr[s][:, h % 2, (j % 4) * P:(j % 4 + 1) * P],
                        rhs=qaug[h][:], start=True, stop=True)

                def attnv_mm(h, o_ps, sc, idx, j):
                    s = ((h // 2) % 2) * 8 + j // 4
                    hi = h % 2
                    nc.tensor.matmul(
                        o_ps[hi * DH:(hi + 1) * DH, :],
                        lhsT=vPr[s][:, j % 4, hi * DH:(hi + 1) * DH],
                        rhs=sc[:, idx * SL:(idx + 1) * SL],
                        start=(j == 0), stop=(j == 31),
                        tile_position=(0, hi * DH))

                def emit_proj_chunk(m, nt):
                    """out-proj: acc_sb[nt] (+)= (Wo^T O^T)[nt] chunk m."""
                    psc = psS.tile([P, SL], f32, name=f"psc{m}_{nt}",
                                   tag="scr")
                    nc.tensor.matmul(psc[:],
                                     lhsT=wo_sb[m][:, nt * P:(nt + 1) * P],
                                     rhs=ot_sb[m][:], start=True, stop=True)
                    if m == 0:
                        nc.vector.tensor_copy(acc_sb[nt][:], psc[:])
                    else:
                        nc.vector.tensor_add(acc_sb[nt][:], acc_sb[nt][:],
                                             psc[:])

                kg_load(0)
                vP_load(0)
                sc_live = {}    # h -> list of sc tiles
                o_live = {}     # pair -> o_ps tile
                for h in range(H):
                    p = h // 2
                    if h % 2 == 0 and p + 1 < NPAIR:
                        kg_load(p + 1)
                    if h % 2 == 1 and (h + 1) // 2 < NPAIR:
                        vP_load((h + 1) // 2)
                    if h % 2 == 1:
                        # first needed by attn@V(h-1) in this head slot
                        o_live[p] = psO.tile([P, SL], f32, name=f"o_ps{p}",
                                             tag="o_ps")
                    # phase-C interleave: pair m's output-proj chunks ride
                    # in head slots 2m+3 and 2m+4
                    pc = []
                    if h >= 3 and h % 2 == 1:
                        pc = [((h - 3) // 2, nt) for nt in range(3)]
                    elif h >= 4 and h % 2 == 0:
                        pc = [((h - 4) // 2, nt) for nt in range(3, KC)]
                    sc_live[h] = []
                    hprev = h - 1
                    for gi, grp in enumerate(groups):
                        w = len(grp) * SL
                        if hprev >= 0:
                            scp_prev = sc_live[hprev][gi]
                            for idx, j in enumerate(grp):
                                attnv_mm(hprev, o_live[hprev // 2], scp_prev,
                                         idx, j)
                        dist = psD.tile([P, 3 * SL], f32,
                                        name=f"dist{h}_{grp[0]}", tag="dist")
                        for idx, j in enumerate(grp):
                            dist_mm(h, dist, idx, j)
                        sc = scp.tile([P, 3 * SL], fb, name=f"sc{h}_{grp[0]}",
                                      tag="sc")
                        nc.scalar.activation(
                            sc[:, :w], dist[:, :w],
                            mybir.ActivationFunctionType.Exp,
                            scale=float(neg_a[h]))
                        sc_live[h].append(sc)
                        if hprev >= 0 and gi == NG - 1 and hprev % 2 == 1:
                            # pair hprev//2 finished: stage for out-proj
                            nc.vector.tensor_copy(ot_sb[hprev // 2][:],
                                                  o_live[hprev // 2][:])
                        if gi in (3, 6, 9) and pc:
                            emit_proj_chunk(*pc.pop(0))
                    sc_live.pop(h - 2, None)

                # epilogue: attn@V for head 11, last pair copy, out-proj
                # chunks m=4 (nt 3..5) and m=5, store
                h = H - 1
                for gi, grp in enumerate(groups):
                    for idx, j in enumerate(grp):
                        attnv_mm(h, o_live[5], sc_live[h][gi], idx, j)
                nc.vector.tensor_copy(ot_sb[5][:], o_live[5][:])
                for nt in range(3, KC):
                    emit_proj_chunk(4, nt)
                for nt in range(KC):
                    psc = psD.tile([P, SL], f32, name=f"psc5_{nt}", tag="dist")
                    nc.tensor.matmul(psc[:],
                                     lhsT=wo_sb[KC - 1][:, nt * P:(nt + 1) * P],
                                     rhs=ot_sb[KC - 1][:], start=True,
                                     stop=True)
                    nc.vector.tensor_add(acc_sb[nt][:], acc_sb[nt][:], psc[:])
                    nc.gpsimd.dma_start(outT[nt * P:(nt + 1) * P, :],
                                        acc_sb[nt][:])

    nc.compile()
    return nc


def prepare_in_maps(x, Wq, Wk, Wv, Wo):
    xT = np.ascontiguousarray(x.reshape(S, D).T)  # [768, 4096]
    wqb = Wq.astype(_BF16)
    wkb = Wk.astype(_BF16)
    wvb = Wv.astype(_BF16)
    wob = Wo.astype(_BF16)
    in_maps = []
    for c in range(N_CORES):
        in_maps.append({
            "xT": np.ascontiguousarray(xT[:, c * SL:(c + 1) * SL]).astype(_BF16),
            "wq": wqb, "wk": wkb, "wv": wvb, "wo": wob,
        })
    return in_maps


def postprocess(results):
    out = np.empty((S, D), np.float32)
    for c in range(N_CORES):
        out[c * SL:(c + 1) * SL, :] = results[c]["outT"].T
    return out.reshape(1, S, D)


_CACHE = {}


def _get_nc(gamma):
    key = tuple(np.asarray(gamma, np.float64).tolist())
    if key not in _CACHE:
        neg_a = [-float(g) * SCALE for g in gamma]
        _CACHE[key] = build(neg_a)
    return _CACHE[key]


def kernel(x, Wq, Wk, Wv, Wo, gamma):
    from concourse.bass_utils import run_bass_kernel_spmd

    x = np.asarray(x, np.float32)
    nc = _get_nc(np.asarray(gamma, np.float32))
    in_maps = prepare_in_maps(x, np.asarray(Wq, np.float32),
                              np.asarray(Wk, np.float32),
                              np.asarray(Wv, np.float32),
                              np.asarray(Wo, np.float32))
    res = run_bass_kernel_spmd(nc, in_maps, core_ids=list(range(N_CORES)))
    return postprocess(res.results)


# revision 20
# speedup vs baseline: 1.1083x; 1.1083x over previous
"""RBF-kernel attention (dense_transformer) on 8 TRN2 NeuronCores.

Reference computation (B=1, S=4096, D=768, H=12, Dh=64):
    q,k,v = x@Wq, x@Wk, x@Wv               (per-head split)
    dist  = ||q_s - k_t||^2
    scores= exp(-gamma_h/8 * dist)
    out   = (scores @ v) merged @ Wo

Sharding: 8-way data parallel over query rows (512 rows/core).  Each core
computes its local K/V shard + per-head k-norms, all-gathers an augmented
K (rows: [k(64); kn_hi; kn_lo]) and V across cores, then computes the
full distance matrix for its queries with a single 80-deep matmul per
tile (rows 66:80 = [1;1;0*12] are constant and reconstructed on-chip):
    dist[t,s] = kaug[:,t] . qaug[:,s],  qaug = [-2q; 1; 1; qn_hi; qn_lo; 0]
(contraction padded to 80: K%16 != 0 streams at half rate on the PE).

v3 schedule: the first collective cannot execute before ~65us after
launch (fixed CC warmup) and each mesh costs ~10us fixed + bytes/190GB/s,
so the gathers are packed into 7 parts ordered by phase-B deadline:
m0={kaug pair0}, m_i={V pair i-1, kaug pair i} (i=1..5), m6={V pair5}.
All Q projections run in the otherwise-idle pre-gather window.  Phase B
is software-pipelined by head: dist/exp for head h interleaves with
attn@V for head h-1, so the V gather deadline trails the kaug deadline
by a full head slot (~16us).  The output projection is interleaved into
later head slots (SBUF accumulation via DVE adds off a single scratch
PSUM bank), so there is no serial projection tail.  The two heads of a
pair share one PSUM bank for attn@V output (odd head at partition
offset 64 via tile_position).  Norm matmuls run in bf16 (fp32 operands
cost 4 cycles/row on the PE).  exp runs on the scalar engine straight
out of PSUM with the per-head scale folded in, over 1536-column groups
to amortize ACT overhead.  attn@V is computed transposed (out^T[d,s])
so no on-chip transposes are needed, and the final Wo matmul emits the
core's output slice transposed ([768, 512]); the host transposes and
concatenates.  All TensorE-facing data is bf16 (fp32 PSUM accumulation);
k/q norms get a hi+lo bf16 split so the exponent stays fp32-accurate.
"""

import numpy as np
import ml_dtypes

N_CORES = 8
S = 4096          # sequence length
D = 768           # embed dim
H = 12            # heads
DH = 64           # head dim
SL = S // N_CORES # query rows per core (512)
P = 128
KC = D // P       # contraction chunks for projections (6)
NAUG = DH + 4     # meaningful aug rows (68)
AUG = 80          # padded to mult-of-16: K%16!=0 matmuls stream at half rate
GAUG = DH + 2     # gathered aug rows (66): k + kn_hi + kn_lo
SCALE = 1.0 / np.sqrt(DH)
NPAIR = H // 2    # 6 head pairs

_BF16 = ml_dtypes.bfloat16


def build(neg_a):
    """Build the SPMD Bass graph. neg_a: list of 12 floats (-gamma[h]*SCALE)."""
    import concourse.bass as bass  # noqa: F401
    import concourse.mybir as mybir
    import concourse.tile as tile
    from concourse import bacc

    fb = mybir.dt.bfloat16
    f32 = mybir.dt.float32

    nc = bacc.Bacc("TRN2", target_bir_lowering=False, debug=False,
                   num_devices=N_CORES)

    xT = nc.dram_tensor("xT", [D, SL], fb, kind="ExternalInput").ap()
    wq = nc.dram_tensor("wq", [D, D], fb, kind="ExternalInput").ap()
    wk = nc.dram_tensor("wk", [D, D], fb, kind="ExternalInput").ap()
    wv = nc.dram_tensor("wv", [D, D], fb, kind="ExternalInput").ap()
    wo = nc.dram_tensor("wo", [D, D], fb, kind="ExternalInput").ap()
    outT = nc.dram_tensor("outT", [D, SL], f32, kind="ExternalOutput").ap()

    # 7 sub-1MB gathers ordered by phase-B deadline (V pair0 rides with
    # kaug pair0 in m0 -- the ~65us collective-warmup floor hides the
    # later V-projection finish, and head0's attn@V then never stalls):
    #   part 0      = kaug pair0 + V pair0        (266KB send)
    #   part 1      = kaug pair1                  (135KB send)
    #   part 2..5   = V pair p-1 + kaug pair p    (266KB send)
    #   part 6      = V pair5                     (131KB send)
    KSZ = 2 * GAUG * SL
    VSZ = SL * P
    PART_SZ = [KSZ + VSZ, KSZ] + [KSZ + VSZ] * 4 + [VSZ]
    # V part/offset for head pair hp
    VPART = [(0, KSZ)] + [(hp + 1, KSZ) for hp in range(1, 5)] + [(6, 0)]
    fsend = [nc.dram_tensor(f"fsend{p}", [PART_SZ[p]], fb) for p in range(7)]
    fg = [nc.dram_tensor(f"fg{p}", [N_CORES * PART_SZ[p]], fb,
                         addr_space="Shared") for p in range(7)]
    rg = [list(range(N_CORES))]

    def ksend2d(h, row, nrows):
        base = (h % 2) * GAUG * SL + row * SL
        return fsend[h // 2][base:base + nrows * SL].rearrange(
            "(a b) -> a b", b=SL)

    def vsend2d(hp, tt):
        prt, off = VPART[hp]
        base = off + tt * P * P
        return fsend[prt][base:base + P * P].rearrange("(a b) -> a b", b=P)

    def kg_src(p, c):
        """Gathered kaug of pair p, core c: [GAUG, 2, SL]."""
        base = c * PART_SZ[p]
        return fg[p][base:base + KSZ].rearrange(
            "(h a b) -> a h b", h=2, b=SL)

    def vg_src(hp, c):
        """Gathered V cols of pair hp, keys c*512..: [128, 4, 128]."""
        prt, off = VPART[hp]
        base = c * PART_SZ[prt] + off
        return fg[prt][base:base + VSZ].rearrange(
            "(j p c) -> p j c", j=4, p=P, c=P)

    groups = [list(range(g * 3, min(32, g * 3 + 3)))
              for g in range((32 + 2) // 3)]
    NG = len(groups)  # 11

    with tile.TileContext(nc) as tc:
        with tc.tile_pool(name="persist", bufs=1) as pp:
            xT_sb = [pp.tile([P, SL], fb, name=f"xT_sb{k}") for k in range(KC)]
            wo_sb = [pp.tile([P, D], fb, name=f"wo_sb{k}") for k in range(KC)]
            qaug = [pp.tile([AUG, SL], fb, name=f"qaug{h}") for h in range(H)]
            # 2-pair rings: gathered kaug [80, 2, SL] (rows 66:80 constant,
            # initialized once) and gathered V [128, 4, 128]
            kgr = [pp.tile([AUG, 2, SL], fb, name=f"kgr{s}") for s in range(16)]
            vPr = [pp.tile([P, 4, P], fb, name=f"vPr{s}") for s in range(16)]
            ot_sb = [pp.tile([P, SL], fb, name=f"ot_sb{m}") for m in range(KC)]
            acc_sb = [pp.tile([P, SL], f32, name=f"acc_sb{m}")
                      for m in range(KC)]
            hsel = pp.tile([P, 2], fb, name="hsel")

            # K-path input loads first: the kaug gathers gate phase B.
            for k in range(KC):
                nc.scalar.dma_start(xT_sb[k][:], xT[k * P:(k + 1) * P, :])

            # head-pair selector for partition-sum via matmul:
            # col j sums partitions j*64..j*64+63  (bf16: fp32 matmuls
            # stream at 1/4 rate on the PE)
            nc.vector.memset(hsel[:], 0.0)
            nc.vector.memset(hsel[0:DH, 0:1], 1.0)
            nc.vector.memset(hsel[DH:P, 1:2], 1.0)

            ones_sb = pp.tile([2, SL], fb, name="ones_sb")
            nc.vector.memset(ones_sb[:], 1.0)
            zeros_sb = pp.tile([AUG - NAUG, SL], fb, name="zeros_sb")
            nc.vector.memset(zeros_sb[:], 0.0)
            # kaug rows 66:80 = [1,1,0*12] in both head slots: one-time init
            onz_sb = pp.tile([AUG - GAUG, 2 * SL], fb, name="onz_sb")
            nc.vector.memset(onz_sb[:], 0.0)
            nc.vector.memset(onz_sb[0:2, :], 1.0)

            def project_T(w_sb, dt, pool):
                """psum[128, SL] = (W^T x^T) rows dt*128..+128."""
                ps = pool.tile([P, SL], f32, name=f"projT{dt}", tag="scr")
                for k in range(KC):
                    nc.tensor.matmul(ps[:], lhsT=w_sb[k][:, dt * P:(dt + 1) * P],
                                     rhs=xT_sb[k][:], start=(k == 0),
                                     stop=(k == KC - 1))
                return ps

            def norms(ps_bf, dt, tag, wpool, npool):
                """hi/lo bf16 split of per-head sum of squares.

                Returns [34, SL] tile: rows 0:2 = hi (head pair), rows
                32:34 = lo -- 32-aligned so compute engines may write both,
                and nhl[half::32] DMAs one head's (hi, lo) pair at once.
                """
                sq = wpool.tile([P, SL], fb, name=f"sq_{tag}{dt}",
                                tag=f"sq{tag}")
                nc.vector.tensor_mul(sq[:], ps_bf[:], ps_bf[:])
                nps = npool.tile([2, SL], f32, name=f"n_{tag}{dt}", tag="nrm")
                nc.tensor.matmul(nps[:], lhsT=hsel[:], rhs=sq[:],
                                 start=True, stop=True)
                nhl = wpool.tile([34, SL], fb, name=f"nhl_{tag}{dt}",
                                 tag=f"nhl{tag}")
                nc.vector.tensor_copy(nhl[0:2, :], nps[:])
                nc.vector.tensor_sub(nhl[32:34, :], nps[:], nhl[0:2, :])
                return nhl

            # ---------------- phase A -------------------------------------
            with tc.tile_pool(name="psA", bufs=3, space="PSUM") as psA, \
                 tc.tile_pool(name="psN", bufs=2, space="PSUM") as psN, \
                 tc.tile_pool(name="workA", bufs=3) as wa:

                wk_sb = [wa.tile([P, D], fb, name=f"wk_sb{k}", bufs=1)
                         for k in range(KC)]
                wv_sb = [wa.tile([P, D], fb, name=f"wv_sb{k}", bufs=1)
                         for k in range(KC)]
                wq_sb = [wa.tile([P, D], fb, name=f"wq_sb{k}", bufs=1)
                         for k in range(KC)]
                # wk dt0/dt1 column pieces first, then wv (V projections
                # start right after K dt0/dt1), then wq, then the rest
                for dt in range(2):
                    for k in range(KC):
                        nc.sync.dma_start(
                            wk_sb[k][:, dt * P:(dt + 1) * P],
                            wk[k * P:(k + 1) * P, dt * P:(dt + 1) * P])
                for k in range(KC):
                    nc.sync.dma_start(wv_sb[k][:], wv[k * P:(k + 1) * P, :])
                for k in range(KC):
                    nc.sync.dma_start(wq_sb[k][:], wq[k * P:(k + 1) * P, :])
                for dt in range(2, KC):
                    for k in range(KC):
                        nc.sync.dma_start(
                            wk_sb[k][:, dt * P:(dt + 1) * P],
                            wk[k * P:(k + 1) * P, dt * P:(dt + 1) * P])
                for k in range(KC):
                    nc.sync.dma_start(wo_sb[k][:], wo[k * P:(k + 1) * P, :])

                # one-time kaug constant rows (66:80 = [1,1,0*12]); early on
                # the sync queue so they never sit between the Q-side aug
                # DMAs and the first kaug gather loads
                for s in range(16):
                    nc.sync.dma_start(
                        kgr[s][GAUG:AUG, :, :],
                        onz_sb[:].rearrange("a (h b) -> a h b", h=2))

                def emit_k_dt(dt):
                    """K projection chunk dt -> kaug sends (66 rows)."""
                    ps = project_T(wk_sb, dt, psA)
                    ktb = wa.tile([P, SL], fb, name=f"ktb{dt}", tag="ktb")
                    nc.vector.tensor_copy(ktb[:], ps[:])
                    nhl = norms(ktb, dt, "k", wa, psN)
                    for half in range(2):
                        h = 2 * dt + half
                        nc.scalar.dma_start(ksend2d(h, 0, DH),
                                            ktb[half * DH:(half + 1) * DH, :])
                        nc.scalar.dma_start(ksend2d(h, DH, 2),
                                            nhl[half:34:32, :])

                def fire(p):
                    nc.gpsimd.collective_compute(
                        "AllGather", mybir.AluOpType.bypass,
                        ins=[fsend[p][:]], outs=[fg[p][:]], replica_groups=rg)

                emit_k_dt(0)
                emit_k_dt(1)

                # V local (natural layout), sends per head-pair column block
                for tt in range(SL // P):
                    vloc = wa.tile([P, D], fb, name=f"vloc{tt}", tag="vloc")
                    for nh in range(2):
                        ps = psA.tile([P, 384], f32, name=f"vps{tt}_{nh}",
                                      tag="scr")
                        for k in range(KC):
                            nc.tensor.matmul(
                                ps[:], lhsT=xT_sb[k][:, tt * P:(tt + 1) * P],
                                rhs=wv_sb[k][:, nh * 384:(nh + 1) * 384],
                                start=(k == 0), stop=(k == KC - 1))
                        nc.vector.tensor_copy(vloc[:, nh * 384:(nh + 1) * 384],
                                              ps[:])
                    for hp in range(NPAIR):
                        nc.scalar.dma_start(vsend2d(hp, tt),
                                            vloc[:, hp * P:(hp + 1) * P])

                fire(0)
                fire(1)
                emit_k_dt(2)
                fire(2)
                emit_k_dt(3)
                fire(3)
                emit_k_dt(4)
                fire(4)
                emit_k_dt(5)
                fire(5)
                fire(6)

                # all Q projections fill the pre-gather idle window.  The
                # qtb/nhl tags are distinct from the K side so buffer reuse
                # never makes the Q pipeline wait on K send DMAs.
                for dt in range(KC):
                    ps = project_T(wq_sb, dt, psA)
                    qtb = wa.tile([P, SL], fb, name=f"qtb{dt}", tag="qtb")
                    nc.vector.tensor_copy(qtb[:], ps[:])
                    nhl = norms(qtb, dt, "q", wa, psN)
                    for half in range(2):
                        h = 2 * dt + half
                        qa = qaug[h]
                        nc.vector.tensor_scalar_mul(
                            qa[0:DH, :], qtb[half * DH:(half + 1) * DH, :],
                            -2.0)
                        # rows 64-67 ([1;1;qn_hi;qn_lo]) + zero pad via DMA:
                        # partition offsets 65..67 aren't 32-aligned for
                        # compute engines.  Scalar queue: on sync these
                        # would head-of-line-block the kaug gather loads
                        # behind the last nhl (vector) dependency.
                        nc.scalar.dma_start(qa[DH:DH + 2, :], ones_sb[:])
                        nc.scalar.dma_start(qa[DH + 2:DH + 4, :],
                                            nhl[half:34:32, :])
                        nc.scalar.dma_start(qa[NAUG:AUG, :], zeros_sb[:])

            # ---------------- phase B: scores + attn@V --------------------
            # software-pipelined by head: head h's dist/exp interleaves
            # with head h-1's attn@V.  PSUM: 2*3 (dist) + 1 (o_ps pair-
            # packed) + 1 scratch (out-proj) = 8 banks
            with tc.tile_pool(name="psD", bufs=2, space="PSUM") as psD, \
                 tc.tile_pool(name="psO", bufs=1, space="PSUM") as psO, \
                 tc.tile_pool(name="psS", bufs=1, space="PSUM") as psS, \
                 tc.tile_pool(name="scp", bufs=16) as scp:

                def kg_load(p):
                    for c in range(N_CORES):
                        s = (p % 2) * 8 + c
                        nc.sync.dma_start(kgr[s][0:GAUG, :, :], kg_src(p, c))

                def vP_load(p):
                    for c in range(N_CORES):
                        s = (p % 2) * 8 + c
                        nc.sync.dma_start(vPr[s][:], vg_src(p, c))

                def dist_mm(h, dist, idx, j):
                    s = ((h // 2) % 2) * 8 + j // 4
                    nc.tensor.matmul(
                        dist[:, idx * SL:(idx + 1) * SL],
                        lhsT=kgr[s][:, h % 2, (j % 4) * P:(j % 4 + 1) * P],
                        rhs=qaug[h][:], start=True, stop=True)

                def attnv_mm(h, o_ps, sc, idx, j):
                    s = ((h // 2) % 2) * 8 + j // 4
                    hi = h % 2
                    nc.tensor.matmul(
                        o_ps[hi * DH:(hi + 1) * DH, :],
                        lhsT=vPr[s][:, j % 4, hi * DH:(hi + 1) * DH],
                        rhs=sc[:, idx * SL:(idx + 1) * SL],
                        start=(j == 0), stop=(j == 31),
                        tile_position=(0, hi * DH))

                def emit_proj_chunk(m, nt):
                    """out-proj: acc_sb[nt] (+)= (Wo^T O^T)[nt] chunk m."""
                    psc = psS.tile([P, SL], f32, name=f"psc{m}_{nt}",
                                   tag="scr")
                    nc.tensor.matmul(psc[:],
                                     lhsT=wo_sb[m][:, nt * P:(nt + 1) * P],
                                     rhs=ot_sb[m][:], start=True, stop=True)
                    if m == 0:
                        nc.vector.tensor_copy(acc_sb[nt][:], psc[:])
                    else:
                        nc.vector.tensor_add(acc_sb[nt][:], acc_sb[nt][:],
                                             psc[:])

                kg_load(0)
                vP_load(0)
                sc_live = {}    # h -> list of sc tiles
                o_live = {}     # pair -> o_ps tile
                for h in range(H):
                    p = h // 2
                    if h % 2 == 0 and p + 1 < NPAIR:
                        kg_load(p + 1)
                    if h % 2 == 1 and (h + 1) // 2 < NPAIR:
                        vP_load((h + 1) // 2)
                    if h % 2 == 1:
                        # first needed by attn@V(h-1) in this head slot
                        o_live[p] = psO.tile([P, SL], f32, name=f"o_ps{p}",
                                             tag="o_ps")
                    # phase-C interleave: pair m's output-proj chunks ride
                    # in head slots 2m+3 and 2m+4
                    pc = []
                    if h >= 3 and h % 2 == 1:
                        pc = [((h - 3) // 2, nt) for nt in range(3)]
                    elif h >= 4 and h % 2 == 0:
                        pc = [((h - 4) // 2, nt) for nt in range(3, KC)]
                    sc_live[h] = []
                    hprev = h - 1
                    for gi, grp in enumerate(groups):
                        w = len(grp) * SL
                        if hprev >= 0:
                            scp_prev = sc_live[hprev][gi]
                            for idx, j in enumerate(grp):
                                attnv_mm(hprev, o_live[hprev // 2], scp_prev,
                                         idx, j)
                        dist = psD.tile([P, 3 * SL], f32,
                                        name=f"dist{h}_{grp[0]}", tag="dist")
                        for idx, j in enumerate(grp):
                            dist_mm(h, dist, idx, j)
                        sc = scp.tile([P, 3 * SL], fb, name=f"sc{h}_{grp[0]}",
                                      tag="sc")
                        nc.scalar.activation(
                            sc[:, :w], dist[:, :w],
                            mybir.ActivationFunctionType.Exp,
                            scale=float(neg_a[h]))
                        sc_live[h].append(sc)
                        if hprev >= 0 and gi == NG - 1 and hprev % 2 == 1:
                            # pair hprev//2 finished: stage for out-proj
                            nc.vector.tensor_copy(ot_sb[hprev // 2][:],
                                                  o_live[hprev // 2][:])
                        if gi in (3, 6, 9) and pc:
                            emit_proj_chunk(*pc.pop(0))
                    sc_live.pop(h - 2, None)

                # epilogue: attn@V for head 11, last pair copy, out-proj
                # chunks m=4 (nt 3..5) and m=5, store
                h = H - 1
                for gi, grp in enumerate(groups):
                    for idx, j in enumerate(grp):
                        attnv_mm(h, o_live[5], sc_live[h][gi], idx, j)
                nc.vector.tensor_copy(ot_sb[5][:], o_live[5][:])
                for nt in range(3, KC):
                    emit_proj_chunk(4, nt)
                for nt in range(KC):
                    psc = psD.tile([P, SL], f32, name=f"psc5_{nt}", tag="dist")
                    nc.tensor.matmul(psc[:],
                                     lhsT=wo_sb[KC - 1][:, nt * P:(nt + 1) * P],
                                     rhs=ot_sb[KC - 1][:], start=True,
                                     stop=True)
                    nc.vector.tensor_add(acc_sb[nt][:], acc_sb[nt][:], psc[:])
                    nc.gpsimd.dma_start(outT[nt * P:(nt + 1) * P, :],
                                        acc_sb[nt][:])

    nc.compile()
    return nc


def prepare_in_maps(x, Wq, Wk, Wv, Wo):
    xT = np.ascontiguousarray(x.reshape(S, D).T)  # [768, 4096]
    wqb = Wq.astype(_BF16)
    wkb = Wk.astype(_BF16)
    wvb = Wv.astype(_BF16)
    wob = Wo.astype(_BF16)
    in_maps = []
    for c in range(N_CORES):
        in_maps.append({
            "xT": np.ascontiguousarray(xT[:, c * SL:(c + 1) * SL]).astype(_BF16),
            "wq": wqb, "wk": wkb, "wv": wvb, "wo": wob,
        })
    return in_maps


def postprocess(results):
    out = np.empty((S, D), np.float32)
    for c in range(N_CORES):
        out[c * SL:(c + 1) * SL, :] = results[c]["outT"].T
    return out.reshape(1, S, D)


_CACHE = {}


def _get_nc(gamma):
    key = tuple(np.asarray(gamma, np.float64).tolist())
    if key not in _CACHE:
        neg_a = [-float(g) * SCALE for g in gamma]
        _CACHE[key] = build(neg_a)
    return _CACHE[key]


def kernel(x, Wq, Wk, Wv, Wo, gamma):
    from concourse.bass_utils import run_bass_kernel_spmd

    x = np.asarray(x, np.float32)
    nc = _get_nc(np.asarray(gamma, np.float32))
    in_maps = prepare_in_maps(x, np.asarray(Wq, np.float32),
                              np.asarray(Wk, np.float32),
                              np.asarray(Wv, np.float32),
                              np.asarray(Wo, np.float32))
    res = run_bass_kernel_spmd(nc, in_maps, core_ids=list(range(N_CORES)))
    return postprocess(res.results)
